# revision 16
# baseline (speedup 1.0000x reference)
"""EntityAttention Trainium2 kernel (nn_EntityAttention_31525059952740).

Math (per (batch, entity) group n, all 64 events e):
  q = (events @ Wq.T + bq) * scale            shared across n     [64, 512]
  k = toks_b @ Wk.T + bk                      per batch           [512, 512]
  v = toks_b @ Wv.T + bv                      per batch           [512, 512]
  scores[h,e,s] = q_h[e] . k_h[s]             per batch (2 heads x 256)
  attn = softmax over s, masked by entities[n]  (mask = multiplicative
         0/1 on exp since scores are tiny, no max-subtraction needed)
  out[e] = concat_h(attn_h @ v_h);  O = out @ Wo.T + bo

Sharding: batch b -> core b (8 batches, 8 cores). Each core computes all 16
entities of its batch -> output rows [1024, 512] per core, concatenated.

v2 (this file): all matmul operands fp16 (full PE rate at any output width,
half the DMA bytes, 2x DVE), wv/wo loaded via Pool-engine SWDGE (parallel
descriptor-gen lane to the global HWDGE), PE p-state pre-ramp with dummy
matmuls during the DMA lead-in, PV/O interleaved per 4-entity group so the
DVE normalization stays off the PE critical path, fp16 output staged and
DMA'd per 128-row pair. Weight-side folds are done on host: wtil = scaled-q
@ Wk (bk cancels in softmax), bv contributes bv @ Wo.T to the output bias.
"""

import numpy as np

import concourse.bass as bass
import concourse.tile as tile
import concourse.mybir as mybir
from concourse import bacc
from concourse.bass_utils import run_bass_kernel_spmd

NB, SL, NH, EN, NE, HEADS = 8, 512, 512, 16, 64, 2
DH = NH // HEADS          # 256
P = 128
NCHUNK = NH // P          # 4 chunks of the hidden dim
SCHUNK = SL // P          # 4 chunks of the sequence dim
SCALE = 1.0 / np.sqrt(DH).astype(np.float32)

F32 = mybir.dt.float32
F16 = mybir.dt.float16

_CACHE = {}


def _build():
    nc = bacc.Bacc("TRN2", target_bir_lowering=False, debug=False, num_devices=NB)

    # ---- I/O (all fp16) ----
    toksT_d = nc.dram_tensor("toksT", [NH, SL], F16, kind="ExternalInput").ap()
    wtil_d = nc.dram_tensor("wtil", [P, NCHUNK, HEADS * NE], F16,
                            kind="ExternalInput").ap()
    masks_d = nc.dram_tensor("masks", [P, SCHUNK, EN], F16,
                             kind="ExternalInput").ap()
    masks32_d = nc.dram_tensor("masks32", [P, SCHUNK, EN], F32,
                               kind="ExternalInput").ap()
    wvT_d = nc.dram_tensor("WvT", [NH, NH], F16, kind="ExternalInput").ap()
    woT_d = nc.dram_tensor("WoT", [NH, NH], F16, kind="ExternalInput").ap()
    out_d = nc.dram_tensor("out", [EN * NE, NH], F16, kind="ExternalOutput").ap()

    EXP = mybir.ActivationFunctionType.Exp
    CPY = mybir.ActivationFunctionType.Copy

    with tile.TileContext(nc) as tc:
        with (
            tc.tile_pool(name="wpool", bufs=1) as wpool,
            tc.tile_pool(name="sb", bufs=1) as sb,
            tc.tile_pool(name="ostage", bufs=2) as ostage,
            tc.tile_pool(name="pps", bufs=1, space="PSUM") as pps,
            tc.tile_pool(name="misc", bufs=1, space="PSUM") as pmisc,
            tc.tile_pool(name="pbig", bufs=6, space="PSUM") as pbig,
            tc.tile_pool(name="dram", bufs=1, space="DRAM") as dram,
        ):
            # ---------- input DMAs ----------
            # HWDGE lane (SP engine): toks halves, wtil, masks.
            toksT_r = toksT_d.rearrange("(c p) s -> p c s", p=P)
            toks01 = wpool.tile([P, 2, SL], F16, tag="toks01")
            nc.sync.dma_start(toks01[:], toksT_r[:, 0:2, :])
            toks23 = wpool.tile([P, 2, SL], F16, tag="toks23")
            nc.sync.dma_start(toks23[:], toksT_r[:, 2:4, :])
            wtil_t = wpool.tile([P, NCHUNK, HEADS * NE], F16, tag="wtil")
            nc.sync.dma_start(wtil_t[:], wtil_d)
            masks_t = wpool.tile([P, SCHUNK, EN], F16, tag="masks")
            nc.sync.dma_start(masks_t[:], masks_d)
            masks32_t = wpool.tile([P, SCHUNK, EN], F32, tag="masks32")
            nc.sync.dma_start(masks32_t[:], masks32_d)

            # SWDGE lane (Pool engine): wv / wo halves.
            wv_sb = wpool.tile([P, NCHUNK, NH], F16, tag="wv")
            wvT_r = wvT_d.rearrange("(c p) d -> p c d", p=P)
            nc.gpsimd.dma_start(wv_sb[:, 0:2, :], wvT_r[:, 0:2, :])
            nc.gpsimd.dma_start(wv_sb[:, 2:4, :], wvT_r[:, 2:4, :])
            wo_sb = wpool.tile([P, NCHUNK, NH], F16, tag="wo")
            woT_r = woT_d.rearrange("(c p) d -> p c d", p=P)
            nc.gpsimd.dma_start(wo_sb[:, 0:2, :], woT_r[:, 0:2, :])
            nc.gpsimd.dma_start(wo_sb[:, 2:4, :], woT_r[:, 2:4, :])

            def toksT(hc):
                return toks01[:, hc, :] if hc < 2 else toks23[:, hc - 2, :]

            masks_sb = masks_t[:]
            wtil_sb = wtil_t[:]

            # ---------- PE p-state pre-ramp ----------
            # ~14 chained dummy matmuls on a zeroed tile keep the PE busy
            # through its frequency ramp while the input DMAs land.
            dummy_sb = sb.tile([P, 256], F16, tag="dummy")
            nc.vector.memset(dummy_sb[:], 0.0)
            pdummy = pmisc.tile([P, 256], F32, tag="pm", name="pdummy")
            NDUM = 14
            for i in range(NDUM):
                nc.tensor.matmul(pdummy[:], dummy_sb[:, 0:P], dummy_sb[:],
                                 start=(i == 0), stop=(i == NDUM - 1))

            # ---------- scores^T = toksT.T @ wtil (fp16, hc-major) ----------
            # sc-major: each accumulation group fully closes before the next
            # one opens — a start=True matmul clears the has_written bits of
            # the WHOLE psum bank, so interleaved groups in one bank corrupt
            # each other.
            pssall = pps.tile([P, SCHUNK, HEADS * NE], F32, tag="pss")
            pss = [pssall[:, sc, :] for sc in range(SCHUNK)]
            for sc in range(SCHUNK):
                for hc in range(NCHUNK):
                    nc.tensor.matmul(
                        pss[sc],
                        toksT(hc)[:, sc * P:(sc + 1) * P], wtil_sb[:, hc, :],
                        start=(hc == 0), stop=(hc == NCHUNK - 1),
                    )
            e_sbs = []
            for sc in range(SCHUNK):
                e_sb = sb.tile([P, HEADS * NE], F16, tag=f"e{sc}")
                nc.scalar.activation(e_sb[:], pss[sc], EXP)
                e_sbs.append(e_sb)

            # ---------- S = masksT.T @ E ; recip; DRAM-roundtrip bcast ------
            pS = pmisc.tile([EN, HEADS * NE], F32, tag="pm", name="pS")
            for sc in range(SCHUNK):
                nc.tensor.matmul(pS[:], masks_sb[:, sc, :], e_sbs[sc][:],
                                 start=(sc == 0), stop=(sc == SCHUNK - 1))
            srec = sb.tile([EN, HEADS * NE], F32, tag="srec")
            nc.vector.reciprocal(srec[:], pS[:])
            srec_dram = dram.tile([EN, HEADS * NE], F32)
            nc.sync.dma_start(srec_dram[:], srec[:])
            srec_bcs = []
            for grp in range(4):
                t = sb.tile([P, 4, HEADS * NE], F32, tag=f"srec_bc{grp}")
                sd_ap = srec_dram[grp * 4:(grp + 1) * 4, :]
                nc.sync.dma_start(
                    t[:],
                    bass.AP(tensor=sd_ap.tensor, offset=sd_ap.offset,
                            ap=[[0, P], *sd_ap.ap]),
                )
                srec_bcs.append(t)

            def srec_slice(grp, h):
                return srec_bcs[grp][:, :, h * NE:(h + 1) * NE]

            # ---------- V = toks @ WvT ----------
            vs = []
            for i in range(SCHUNK):
                pv = pbig.tile([P, NH], F32, tag="pb", name=f"pv{i}")
                for hc in range(NCHUNK):
                    nc.tensor.matmul(
                        pv[:], toksT(hc)[:, i * P:(i + 1) * P], wv_sb[:, hc, :],
                        start=(hc == 0), stop=(hc == NCHUNK - 1),
                    )
                v = sb.tile([P, NH], F16, tag=f"v{i}")
                nc.vector.tensor_copy(v[:], pv[:])
                vs.append(v)

            # ---------- attnT = E * mask (per entity), fp16 ----------
            # engine split per 4-entity group: k=0 -> DVE, k=1,2 -> Act,
            # k=3 -> Pool.
            attnTs = {}
            for grp in range(4):
                for sc in range(SCHUNK):
                    attnT = sb.tile([P, 4, HEADS * NE], F16,
                                    tag=f"attnT{grp}_{sc}")
                    for k in range(4):
                        ent = grp * 4 + k
                        msk = masks32_t[:, sc, ent:ent + 1]
                        if k == 0:
                            nc.vector.tensor_scalar_mul(
                                attnT[:, k, :], e_sbs[sc][:], msk)
                        elif k == 3:
                            nc.gpsimd.tensor_scalar_mul(
                                attnT[:, k, :], e_sbs[sc][:], msk)
                        else:
                            nc.scalar.activation(
                                attnT[:, k, :], e_sbs[sc][:], CPY, scale=msk)
                    attnTs[(grp, sc)] = attnT

            # ---------- PV + normalize + O, software-pipelined ----------
            # PE order: PV g0, PV g1, O g0, PV g2, O g1, PV g3, O g2, O g3
            # so each group's DVE normalization overlaps the next group's PV.
            outTs = {}

            def emit_pv(grp):
                outT = sb.tile([P, NCHUNK, 4, NE], F16, tag=f"outT{grp}")
                outTs[grp] = outT
                for h in range(HEADS):
                    po2 = pbig.tile([P, 2, 4 * NE], F32, tag="pb",
                                    name=f"pos_{grp}_{h}")
                    for j in range(2):
                        dc = 2 * h + j
                        for sc in range(SCHUNK):
                            nc.tensor.matmul(
                                po2[:, j, :],
                                vs[sc][:, dc * P:(dc + 1) * P],
                                attnTs[(grp, sc)][:, :, h * NE:(h + 1) * NE],
                                start=(sc == 0), stop=(sc == SCHUNK - 1),
                            )
                        # normalize: outT = po * (1/S), f32 psum -> fp16 sbuf
                        # (DVE only: GPSIMD cannot access PSUM)
                        nc.vector.tensor_mul(
                            outT[:, dc, :, :], po2[:, j, :],
                            srec_slice(grp, h))

            def emit_o(grp):
                outT = outTs[grp]
                for lp in range(2):
                    pair = grp * 2 + lp
                    pO = pbig.tile([P, NH], F32, tag="pb", name=f"pO{pair}")
                    for hc in range(NCHUNK):
                        nc.tensor.matmul(
                            pO[:], outT[:, hc, 2 * lp:2 * lp + 2, :],
                            wo_sb[:, hc, :],
                            start=(hc == 0), stop=(hc == NCHUNK - 1),
                        )
                    o_sb = ostage.tile([P, NH], F16, tag="osb",
                                       name=f"osb{pair}")
                    H2 = NH // 2
                    if pair < 4:
                        nc.vector.tensor_copy(o_sb[:], pO[:])
                    else:
                        nc.vector.tensor_copy(o_sb[:, :H2], pO[:, :H2])
                        nc.scalar.activation(o_sb[:, H2:], pO[:, H2:], CPY)
                    nc.sync.dma_start(
                        out_d[pair * P:(pair + 1) * P, :], o_sb[:])

            emit_pv(0)
            emit_pv(1)
            emit_o(0)
            emit_pv(2)
            emit_o(1)
            emit_pv(3)
            emit_o(2)
            emit_o(3)

    nc.compile()
    return nc


def _get_nc():
    if "nc" not in _CACHE:
        _CACHE["nc"] = _build()
    return _CACHE["nc"]


def _fast_run(nc, in_maps):
    """Repeat-call path: same PJRT execution as run_bass_kernel_spmd/
    bass2jax.run_bass_via_pjrt, but with the jitted shard_map cached so
    repeat kernel() calls skip retracing/relowering."""
    import jax
    from jax.sharding import Mesh, PartitionSpec
    from jax.experimental.shard_map import shard_map
    import concourse.mybir as mybir_
    from concourse import bass2jax

    if "runner" not in _CACHE:
        bass2jax.install_neuronx_cc_hook()
        part_name = (nc.partition_id_tensor.name
                     if nc.partition_id_tensor else None)
        in_names, out_names, out_avals = [], [], []
        for alloc in nc.m.functions[0].allocations:
            if not isinstance(alloc, mybir_.MemoryLocationSet):
                continue
            name = alloc.memorylocations[0].name
            if alloc.kind == "ExternalInput":
                if name != part_name:
                    in_names.append(name)
            elif alloc.kind == "ExternalOutput":
                out_names.append(name)
                out_avals.append(jax.core.ShapedArray(
                    tuple(alloc.tensor_shape), mybir_.dt.np(alloc.dtype)))
        n_params = len(in_names)
        all_in_names = in_names + out_names
        if part_name is not None:
            all_in_names = all_in_names + [part_name]

        def _body(*args):
            operands = list(args)
            if part_name is not None:
                operands.append(bass2jax.partition_id_tensor())
            outs = bass2jax._bass_exec_p.bind(
                *operands,
                out_avals=tuple(out_avals),
                in_names=tuple(all_in_names),
                out_names=tuple(out_names),
                lowering_input_output_aliases=(),
                sim_require_finite=True,
                sim_require_nnan=True,
                nc=nc,
            )
            return tuple(outs)

        devices = jax.devices()[:NB]
        mesh = Mesh(np.asarray(devices), ("core",))
        n_outs = len(out_names)
        sharded = jax.jit(
            shard_map(_body, mesh=mesh,
                      in_specs=(PartitionSpec("core"),) * (n_params + n_outs),
                      out_specs=(PartitionSpec("core"),) * n_outs,
                      check_rep=False),
            donate_argnums=tuple(range(n_params, n_params + n_outs)),
            keep_unused=True,
        )
        _CACHE["runner"] = (sharded, in_names, out_names, out_avals)

    sharded, in_names, out_names, out_avals = _CACHE["runner"]
    concat_in = [
        np.concatenate([np.asarray(m[name]) for m in in_maps], axis=0)
        for name in in_names
    ]
    concat_zeros = [
        np.zeros((NB * av.shape[0], *av.shape[1:]), av.dtype)
        for av in out_avals
    ]
    out_arrs = sharded(*concat_in, *concat_zeros)
    return [
        {name: np.asarray(out_arrs[i]).reshape(NB, *out_avals[i].shape)[c]
         for i, name in enumerate(out_names)}
        for c in range(NB)
    ]


def kernel(tokens_embed, entities, events_embed, entity_num, entity_masks,
           select_event, Wq, Wk, Wv, bq, bk, bv, Wo, bo):
    tokens_embed = np.asarray(tokens_embed, dtype=np.float32)
    entities = np.asarray(entities)
    events_embed = np.asarray(events_embed, dtype=np.float32)
    entity_masks = np.asarray(entity_masks)
    select_event = np.asarray(select_event)
    Wq = np.asarray(Wq, dtype=np.float32)
    Wk = np.asarray(Wk, dtype=np.float32)
    Wv = np.asarray(Wv, dtype=np.float32)
    Wo = np.asarray(Wo, dtype=np.float32)
    bq = np.asarray(bq, dtype=np.float32)
    bk = np.asarray(bk, dtype=np.float32)
    bv = np.asarray(bv, dtype=np.float32)
    bo = np.asarray(bo, dtype=np.float32)

    nc = _get_nc()

    q_s = (events_embed @ Wq.T + bq) * SCALE          # [NE, NH]
    # fold the K projection into the query side (bk cancels in softmax):
    # wtil[hid, (h,e)] = sum_dout_in_head Wk[dout, hid] * q_s[e, dout]
    wtil = np.empty((NH, HEADS * NE), dtype=np.float32)
    for h in range(HEADS):
        hs = slice(h * DH, (h + 1) * DH)
        wtil[:, h * NE:(h + 1) * NE] = (q_s[:, hs] @ Wk[hs, :]).T
    wtil_pc = np.ascontiguousarray(
        wtil.reshape(NCHUNK, P, HEADS * NE).transpose(1, 0, 2)).astype(np.float16)
    # attn rows sum to 1, so the bv term of out contributes bv @ Wo.T to O;
    # the whole output bias is applied host-side after the gather.
    bo2 = (bo + bv @ Wo.T).astype(np.float32)
    shared = {
        "wtil": wtil_pc,
        "WvT": np.ascontiguousarray(Wv.T).astype(np.float16),
        "WoT": np.ascontiguousarray(Wo.T).astype(np.float16),
    }
    in_maps = []
    for c in range(NB):
        # masks[p, sc, ent] = entities[c, ent, sc*128 + p]
        m = entities[c].astype(np.float16)            # [EN, SL]
        mT = np.ascontiguousarray(
            m.reshape(EN, SCHUNK, P).transpose(2, 1, 0))
        in_maps.append({
            "toksT": np.ascontiguousarray(tokens_embed[c].T).astype(np.float16),
            "masks": mT,
            "masks32": mT.astype(np.float32),
            **shared,
        })

    if "ran_once" not in _CACHE:
        res = run_bass_kernel_spmd(nc, in_maps, core_ids=list(range(NB)))
        results = res.results
        _CACHE["ran_once"] = True
    else:
        results = _fast_run(nc, in_maps)
    full = np.concatenate(
        [results[c]["out"].astype(np.float32) for c in range(NB)], axis=0)
    full += bo2[None, :]
    # full[(b*EN + ent)*NE + e] = attention output for group (b, ent), event e

    # ragged selection (mirrors the reference indexing; identity for the
    # all-ones masks produced by setup_inputs)
    assert int(entity_num) == EN
    entity_index = np.flatnonzero(entity_masks.reshape(-1))
    pair_sel = (select_event[:, None, :] & entity_masks[:, :, None])
    pair_sel = pair_sel.reshape(-1, NE)[entity_index].reshape(-1)
    event_entity_index = np.flatnonzero(pair_sel)

    sel_rows = (entity_index[:, None] * NE + np.arange(NE)[None, :]).reshape(-1)
    return full[sel_rows][event_entity_index]


# revision 21
# speedup vs baseline: 1.0808x; 1.0808x over previous
"""EntityAttention Trainium2 kernel (nn_EntityAttention_31525059952740).

Math (per (batch, entity) group n, all 64 events e):
  q = (events @ Wq.T + bq) * scale            shared across n     [64, 512]
  k = toks_b @ Wk.T + bk                      per batch           [512, 512]
  v = toks_b @ Wv.T + bv                      per batch           [512, 512]
  scores[h,e,s] = q_h[e] . k_h[s]             per batch (2 heads x 256)
  attn = softmax over s, masked by entities[n]  (mask = multiplicative
         0/1 on exp since scores are tiny, no max-subtraction needed)
  out[e] = concat_h(attn_h @ v_h);  O = out @ Wo.T + bo

Sharding: batch b -> core b (8 batches, 8 cores). Each core computes all 16
entities of its batch -> output rows [1024, 512] per core, concatenated.

v2 (this file): all matmul operands fp16 (full PE rate at any output width,
half the DMA bytes, 2x DVE), wv/wo loaded via Pool-engine SWDGE (parallel
descriptor-gen lane to the global HWDGE), PE p-state pre-ramp with dummy
matmuls during the DMA lead-in, PV/O interleaved per 4-entity group so the
DVE normalization stays off the PE critical path, fp16 output staged and
DMA'd per 128-row pair. Weight-side folds are done on host: wtil = scaled-q
@ Wk (bk cancels in softmax), bv contributes bv @ Wo.T to the output bias.
"""

import numpy as np

import concourse.bass as bass
import concourse.tile as tile
import concourse.mybir as mybir
from concourse import bacc
from concourse.bass_utils import run_bass_kernel_spmd

NB, SL, NH, EN, NE, HEADS = 8, 512, 512, 16, 64, 2
DH = NH // HEADS          # 256
P = 128
NCHUNK = NH // P          # 4 chunks of the hidden dim
SCHUNK = SL // P          # 4 chunks of the sequence dim
SCALE = 1.0 / np.sqrt(DH).astype(np.float32)

F32 = mybir.dt.float32
F16 = mybir.dt.float16

_CACHE = {}


def _build():
    nc = bacc.Bacc("TRN2", target_bir_lowering=False, debug=False, num_devices=NB)

    # ---- I/O (all fp16) ----
    toksT_d = nc.dram_tensor("toksT", [NH, SL], F16, kind="ExternalInput").ap()
    wtil_d = nc.dram_tensor("wtil", [P, NCHUNK, HEADS * NE], F16,
                            kind="ExternalInput").ap()
    masks_d = nc.dram_tensor("masks", [P, SCHUNK, EN], F16,
                             kind="ExternalInput").ap()
    masks32_d = nc.dram_tensor("masks32", [P, SCHUNK, EN], F32,
                               kind="ExternalInput").ap()
    wvT_d = nc.dram_tensor("WvT", [NH, NH], F16, kind="ExternalInput").ap()
    woT_d = nc.dram_tensor("WoT", [NH, NH], F16, kind="ExternalInput").ap()
    out_d = nc.dram_tensor("out", [EN * NE, NH], F16, kind="ExternalOutput").ap()

    EXP = mybir.ActivationFunctionType.Exp
    CPY = mybir.ActivationFunctionType.Copy

    with tile.TileContext(nc) as tc:
        with (
            tc.tile_pool(name="wpool", bufs=1) as wpool,
            tc.tile_pool(name="sb", bufs=1) as sb,
            tc.tile_pool(name="ostage", bufs=2) as ostage,
            tc.tile_pool(name="pps", bufs=1, space="PSUM") as pps,
            tc.tile_pool(name="misc", bufs=1, space="PSUM") as pmisc,
            tc.tile_pool(name="pbig", bufs=6, space="PSUM") as pbig,
            tc.tile_pool(name="dram", bufs=1, space="DRAM") as dram,
        ):
            # ---------- input DMAs ----------
            # HWDGE lane (SP engine): wtil first (small, gates scores), then
            # toks halves, masks.
            toksT_r = toksT_d.rearrange("(c p) s -> p c s", p=P)
            wtil_t = wpool.tile([P, NCHUNK, HEADS * NE], F16, tag="wtil")
            nc.sync.dma_start(wtil_t[:], wtil_d)
            toks01 = wpool.tile([P, 2, SL], F16, tag="toks01")
            nc.sync.dma_start(toks01[:], toksT_r[:, 0:2, :])
            toks23 = wpool.tile([P, 2, SL], F16, tag="toks23")
            nc.sync.dma_start(toks23[:], toksT_r[:, 2:4, :])
            masks_t = wpool.tile([P, SCHUNK, EN], F16, tag="masks")
            nc.sync.dma_start(masks_t[:], masks_d)
            masks32_t = wpool.tile([P, SCHUNK, EN], F32, tag="masks32")
            nc.sync.dma_start(masks32_t[:], masks32_d)

            # SWDGE lane (Pool engine): wv / wo halves.
            wv_sb = wpool.tile([P, NCHUNK, NH], F16, tag="wv")
            wvT_r = wvT_d.rearrange("(c p) d -> p c d", p=P)
            nc.gpsimd.dma_start(wv_sb[:, 0:2, :], wvT_r[:, 0:2, :])
            nc.gpsimd.dma_start(wv_sb[:, 2:4, :], wvT_r[:, 2:4, :])
            wo_sb = wpool.tile([P, NCHUNK, NH], F16, tag="wo")
            woT_r = woT_d.rearrange("(c p) d -> p c d", p=P)
            nc.gpsimd.dma_start(wo_sb[:, 0:2, :], woT_r[:, 0:2, :])
            nc.gpsimd.dma_start(wo_sb[:, 2:4, :], woT_r[:, 2:4, :])

            def toksT(hc):
                return toks01[:, hc, :] if hc < 2 else toks23[:, hc - 2, :]

            masks_sb = masks_t[:]
            wtil_sb = wtil_t[:]

            # ---------- PE p-state pre-ramp ----------
            # ~14 chained dummy matmuls on a zeroed tile keep the PE busy
            # through its frequency ramp while the input DMAs land.
            dummy_sb = sb.tile([P, 256], F16, tag="dummy")
            nc.vector.memset(dummy_sb[:], 0.0)
            pdummy = pmisc.tile([P, 256], F32, tag="pm", name="pdummy")
            NDUM = 16
            for i in range(NDUM):
                nc.tensor.matmul(pdummy[:], dummy_sb[:, 0:P], dummy_sb[:],
                                 start=(i == 0), stop=(i == NDUM - 1))

            # ---------- scores^T = toksT.T @ wtil (fp16, hc-major) ----------
            # sc-major: each accumulation group fully closes before the next
            # one opens — a start=True matmul clears the has_written bits of
            # the WHOLE psum bank, so interleaved groups in one bank corrupt
            # each other.
            pssall = pps.tile([P, SCHUNK, HEADS * NE], F32, tag="pss")
            pss = [pssall[:, sc, :] for sc in range(SCHUNK)]
            for sc in range(SCHUNK):
                for hc in range(NCHUNK):
                    nc.tensor.matmul(
                        pss[sc],
                        toksT(hc)[:, sc * P:(sc + 1) * P], wtil_sb[:, hc, :],
                        start=(hc == 0), stop=(hc == NCHUNK - 1),
                    )
            e_sbs = []
            for sc in range(SCHUNK):
                e_sb = sb.tile([P, HEADS * NE], F16, tag=f"e{sc}")
                nc.scalar.activation(e_sb[:], pss[sc], EXP)
                e_sbs.append(e_sb)

            # ---------- S = masksT.T @ E ; recip; DRAM-roundtrip bcast ------
            pS = pmisc.tile([EN, HEADS * NE], F32, tag="pm", name="pS")
            for sc in range(SCHUNK):
                nc.tensor.matmul(pS[:], masks_sb[:, sc, :], e_sbs[sc][:],
                                 start=(sc == 0), stop=(sc == SCHUNK - 1))
            srec = sb.tile([EN, HEADS * NE], F32, tag="srec")
            nc.vector.reciprocal(srec[:], pS[:])
            srec_dram = dram.tile([EN, HEADS * NE], F32)
            nc.sync.dma_start(srec_dram[:], srec[:])
            srec_bcs = []
            for grp in range(4):
                t = sb.tile([P, 4, HEADS * NE], F32, tag=f"srec_bc{grp}")
                sd_ap = srec_dram[grp * 4:(grp + 1) * 4, :]
                nc.sync.dma_start(
                    t[:],
                    bass.AP(tensor=sd_ap.tensor, offset=sd_ap.offset,
                            ap=[[0, P], *sd_ap.ap]),
                )
                srec_bcs.append(t)

            def srec_slice(grp, h):
                return srec_bcs[grp][:, :, h * NE:(h + 1) * NE]

            # ---------- attnT = E * mask (per entity), fp16 ----------
            # per-group engine: g0/g1 -> DVE (94ns/op with 4x fp16 mode),
            # g2 -> Pool, g3 -> Act (both ~280ns/op but idle then). g1 is
            # emitted after the V copies so the DVE drains V first.
            attnTs = {}

            def emit_attn(grp, eng):
                for sc in range(SCHUNK):
                    attnT = sb.tile([P, 4, HEADS * NE], F16,
                                    tag=f"attnT{grp}_{sc}")
                    for k in range(4):
                        ent = grp * 4 + k
                        msk = masks32_t[:, sc, ent:ent + 1]
                        if eng == "act":
                            nc.scalar.activation(
                                attnT[:, k, :], e_sbs[sc][:], CPY, scale=msk)
                        elif eng == "pool":
                            nc.gpsimd.tensor_scalar_mul(
                                attnT[:, k, :], e_sbs[sc][:], msk)
                        else:
                            nc.vector.tensor_scalar_mul(
                                attnT[:, k, :], e_sbs[sc][:], msk)
                    attnTs[(grp, sc)] = attnT

            emit_attn(0, "dve")
            emit_attn(2, "pool")
            emit_attn(3, "act")

            # ---------- V = toks @ WvT ----------
            vs = []
            for i in range(SCHUNK):
                pv = pbig.tile([P, NH], F32, tag="pb", name=f"pv{i}")
                for hc in range(NCHUNK):
                    nc.tensor.matmul(
                        pv[:], toksT(hc)[:, i * P:(i + 1) * P], wv_sb[:, hc, :],
                        start=(hc == 0), stop=(hc == NCHUNK - 1),
                    )
                v = sb.tile([P, NH], F16, tag=f"v{i}")
                nc.vector.tensor_copy(v[:], pv[:])
                vs.append(v)

            emit_attn(1, "dve")

            # ---------- PV + normalize + O, software-pipelined ----------
            # PE order: PV g0, PV g1, O g0, PV g2, O g1, PV g3, O g2, O g3
            # so each group's DVE normalization overlaps the next group's PV.
            outTs = {}

            def emit_pv(grp):
                outT = sb.tile([P, NCHUNK, 4, NE], F16, tag=f"outT{grp}")
                outTs[grp] = outT
                for h in range(HEADS):
                    po2 = pbig.tile([P, 2, 4 * NE], F32, tag="pb",
                                    name=f"pos_{grp}_{h}")
                    for j in range(2):
                        dc = 2 * h + j
                        for sc in range(SCHUNK):
                            nc.tensor.matmul(
                                po2[:, j, :],
                                vs[sc][:, dc * P:(dc + 1) * P],
                                attnTs[(grp, sc)][:, :, h * NE:(h + 1) * NE],
                                start=(sc == 0), stop=(sc == SCHUNK - 1),
                            )
                    # normalize: outT = po * (1/S), one DVE op per psum tile
                    # (GPSIMD cannot access PSUM). The srec factor is
                    # j-independent: broadcast with a 0-stride dim.
                    ss = srec_slice(grp, h)
                    ss_b = bass.AP(tensor=ss.tensor, offset=ss.offset,
                                   ap=[ss.ap[0], [0, 2], *ss.ap[1:]])
                    nc.vector.tensor_mul(
                        outT[:, 2 * h:2 * h + 2, :, :], po2[:], ss_b)

            def emit_o(grp):
                outT = outTs[grp]
                for lp in range(2):
                    pair = grp * 2 + lp
                    pO = pbig.tile([P, NH], F32, tag="pb", name=f"pO{pair}")
                    for hc in range(NCHUNK):
                        nc.tensor.matmul(
                            pO[:], outT[:, hc, 2 * lp:2 * lp + 2, :],
                            wo_sb[:, hc, :],
                            start=(hc == 0), stop=(hc == NCHUNK - 1),
                        )
                    o_sb = ostage.tile([P, NH], F16, tag="osb",
                                       name=f"osb{pair}")
                    nc.scalar.activation(o_sb[:], pO[:], CPY)
                    nc.sync.dma_start(
                        out_d[pair * P:(pair + 1) * P, :], o_sb[:])

            emit_pv(0)
            emit_pv(1)
            emit_o(0)
            emit_pv(2)
            emit_o(1)
            emit_pv(3)
            emit_o(2)
            emit_o(3)

    nc.compile()
    return nc


def _get_nc():
    if "nc" not in _CACHE:
        _CACHE["nc"] = _build()
    return _CACHE["nc"]


def _fast_run(nc, in_maps):
    """Repeat-call path: same PJRT execution as run_bass_kernel_spmd/
    bass2jax.run_bass_via_pjrt, but with the jitted shard_map cached so
    repeat kernel() calls skip retracing/relowering."""
    import jax
    from jax.sharding import Mesh, PartitionSpec
    from jax.experimental.shard_map import shard_map
    import concourse.mybir as mybir_
    from concourse import bass2jax

    if "runner" not in _CACHE:
        bass2jax.install_neuronx_cc_hook()
        part_name = (nc.partition_id_tensor.name
                     if nc.partition_id_tensor else None)
        in_names, out_names, out_avals = [], [], []
        for alloc in nc.m.functions[0].allocations:
            if not isinstance(alloc, mybir_.MemoryLocationSet):
                continue
            name = alloc.memorylocations[0].name
            if alloc.kind == "ExternalInput":
                if name != part_name:
                    in_names.append(name)
            elif alloc.kind == "ExternalOutput":
                out_names.append(name)
                out_avals.append(jax.core.ShapedArray(
                    tuple(alloc.tensor_shape), mybir_.dt.np(alloc.dtype)))
        n_params = len(in_names)
        all_in_names = in_names + out_names
        if part_name is not None:
            all_in_names = all_in_names + [part_name]

        def _body(*args):
            operands = list(args)
            if part_name is not None:
                operands.append(bass2jax.partition_id_tensor())
            outs = bass2jax._bass_exec_p.bind(
                *operands,
                out_avals=tuple(out_avals),
                in_names=tuple(all_in_names),
                out_names=tuple(out_names),
                lowering_input_output_aliases=(),
                sim_require_finite=True,
                sim_require_nnan=True,
                nc=nc,
            )
            return tuple(outs)

        devices = jax.devices()[:NB]
        mesh = Mesh(np.asarray(devices), ("core",))
        n_outs = len(out_names)
        sharded = jax.jit(
            shard_map(_body, mesh=mesh,
                      in_specs=(PartitionSpec("core"),) * (n_params + n_outs),
                      out_specs=(PartitionSpec("core"),) * n_outs,
                      check_rep=False),
            donate_argnums=tuple(range(n_params, n_params + n_outs)),
            keep_unused=True,
        )
        _CACHE["runner"] = (sharded, in_names, out_names, out_avals)

    sharded, in_names, out_names, out_avals = _CACHE["runner"]
    concat_in = [
        np.concatenate([np.asarray(m[name]) for m in in_maps], axis=0)
        for name in in_names
    ]
    concat_zeros = [
        np.zeros((NB * av.shape[0], *av.shape[1:]), av.dtype)
        for av in out_avals
    ]
    out_arrs = sharded(*concat_in, *concat_zeros)
    return [
        {name: np.asarray(out_arrs[i]).reshape(NB, *out_avals[i].shape)[c]
         for i, name in enumerate(out_names)}
        for c in range(NB)
    ]


def kernel(tokens_embed, entities, events_embed, entity_num, entity_masks,
           select_event, Wq, Wk, Wv, bq, bk, bv, Wo, bo):
    tokens_embed = np.asarray(tokens_embed, dtype=np.float32)
    entities = np.asarray(entities)
    events_embed = np.asarray(events_embed, dtype=np.float32)
    entity_masks = np.asarray(entity_masks)
    select_event = np.asarray(select_event)
    Wq = np.asarray(Wq, dtype=np.float32)
    Wk = np.asarray(Wk, dtype=np.float32)
    Wv = np.asarray(Wv, dtype=np.float32)
    Wo = np.asarray(Wo, dtype=np.float32)
    bq = np.asarray(bq, dtype=np.float32)
    bk = np.asarray(bk, dtype=np.float32)
    bv = np.asarray(bv, dtype=np.float32)
    bo = np.asarray(bo, dtype=np.float32)

    nc = _get_nc()

    q_s = (events_embed @ Wq.T + bq) * SCALE          # [NE, NH]
    # fold the K projection into the query side (bk cancels in softmax):
    # wtil[hid, (h,e)] = sum_dout_in_head Wk[dout, hid] * q_s[e, dout]
    wtil = np.empty((NH, HEADS * NE), dtype=np.float32)
    for h in range(HEADS):
        hs = slice(h * DH, (h + 1) * DH)
        wtil[:, h * NE:(h + 1) * NE] = (q_s[:, hs] @ Wk[hs, :]).T
    wtil_pc = np.ascontiguousarray(
        wtil.reshape(NCHUNK, P, HEADS * NE).transpose(1, 0, 2)).astype(np.float16)
    # attn rows sum to 1, so the bv term of out contributes bv @ Wo.T to O;
    # the whole output bias is applied host-side after the gather.
    bo2 = (bo + bv @ Wo.T).astype(np.float32)
    shared = {
        "wtil": wtil_pc,
        "WvT": np.ascontiguousarray(Wv.T).astype(np.float16),
        "WoT": np.ascontiguousarray(Wo.T).astype(np.float16),
    }
    in_maps = []
    for c in range(NB):
        # masks[p, sc, ent] = entities[c, ent, sc*128 + p]
        m = entities[c].astype(np.float16)            # [EN, SL]
        mT = np.ascontiguousarray(
            m.reshape(EN, SCHUNK, P).transpose(2, 1, 0))
        in_maps.append({
            "toksT": np.ascontiguousarray(tokens_embed[c].T).astype(np.float16),
            "masks": mT,
            "masks32": mT.astype(np.float32),
            **shared,
        })

    if "ran_once" not in _CACHE:
        res = run_bass_kernel_spmd(nc, in_maps, core_ids=list(range(NB)))
        results = res.results
        _CACHE["ran_once"] = True
    else:
        results = _fast_run(nc, in_maps)
    full = np.concatenate(
        [results[c]["out"].astype(np.float32) for c in range(NB)], axis=0)
    full += bo2[None, :]
    # full[(b*EN + ent)*NE + e] = attention output for group (b, ent), event e

    # ragged selection (mirrors the reference indexing; identity for the
    # all-ones masks produced by setup_inputs)
    assert int(entity_num) == EN
    entity_index = np.flatnonzero(entity_masks.reshape(-1))
    pair_sel = (select_event[:, None, :] & entity_masks[:, :, None])
    pair_sel = pair_sel.reshape(-1, NE)[entity_index].reshape(-1)
    event_entity_index = np.flatnonzero(pair_sel)

    sel_rows = (entity_index[:, None] * NE + np.arange(NE)[None, :]).reshape(-1)
    return full[sel_rows][event_entity_index]
